# revision 1
# baseline (speedup 1.0000x reference)
"""Trainium2 Bass kernel for a 16-layer fully-connected chain (matvec per layer).

Computation (reference):
    v = x                       # [2048]
    for i in 0..13:  v = silu(W[i] @ v + b[i])
    out = W[14] @ v + b[14]

Design (8 NeuronCores, one trn2 chip):
  - Layer 0 is computed REDUNDANTLY in full on every core (256 matmul pairs,
    ~7 us, and its 8 MB fp16 weight DMA) -- both hide entirely inside the
    ~50 us ncfw collective-init barrier that stalls the first AllGather
    anyway, eliminating one gather round.
  - Layers 1..14 are row-sharded: core r computes output neurons
    [r*256, (r+1)*256) as two m-tiles of 128, outputs living ACROSS the 128
    partitions (weights-stationary matmuls: lhsT = W^T tile [k=128, m=128],
    rhs = activation column [128,1], fp32 PSUM accumulated over 16 k-tiles).
    Biases are folded into PSUM with a rank-1 matmul (lhsT = bias row
    [1,128], rhs = const 1.0), so the epilogue is one [128,2] sigmoid + one
    tensor-multiply.
  - fp16 weights+activations with per-layer power-of-2 activation scaling
    folded into the weights host-side (activations grow ~3.2x/layer to ~3e7,
    far beyond fp16 range): psum = y_i/S[i+1], sg = sigmoid(S[i+1]*psum),
    v' = psum*sg (silu is not scale-equivariant). Final layer:
    out = S[14]*psum in fp32.
  - All weight slices are DMA'd to SBUF up front and stream behind compute.
  - Inter-layer gather: the [128, 2] half-pair is PE-transposed to [2, 128]
    so every comm DMA is per-partition contiguous (a [128, few-bytes] DMA
    shatters into per-partition 4B packets, ~13 us/layer of queue drain);
    one fp16 AllGather (512 B -> 4 KB) per layer through internal DRAM
    bounce buffers, then a [16,128] -> [128,16] PE transpose back.
"""

import numpy as np

_L = 15        # number of weight matrices
_N = 2048      # neurons per layer
_M = 8         # cores
_SH = _N // _M  # 256 outputs per core = 2 m-tiles of 128

# S[i] = scale of the stored activation entering layer i (v_i = S[i]*v'_i).
_S = [1.0, 0.5, 1.0, 4.0, 8.0, 32.0, 128.0, 256.0, 1024.0, 2048.0,
      8192.0, 16384.0, 65536.0, 262144.0, 524288.0]

# bias_mm column bases (x128): layers 1..14 pairs first, then layer-0 full.
_NBIAS = 28 + 16
_ONES_COL = _NBIAS * 128

_CACHE = {}


def _build():
    import concourse.bacc as bacc
    import concourse.mybir as mybir
    import concourse.tile as tile

    f32 = mybir.dt.float32
    f16 = mybir.dt.float16
    AF = mybir.ActivationFunctionType

    nc = bacc.Bacc("TRN2", target_bir_lowering=False, debug=False,
                   num_devices=_M)

    # full layer-0 weights: col (kt*16 + mo)*128 + pm holds
    # 2*W[0][mo*128+pm, kt*128+pk] at partition pk.
    wt0 = nc.dram_tensor("wt0", [128, 256 * 128], f16, kind="ExternalInput")
    # sharded layers 1..14: per layer, col ((mt*8 + j)*2 + mo)*128 + pm holds
    # W[i][r*256+mo*128+pm, j*256+mt*128+pk] at partition pk (pre-scaled).
    wt = nc.dram_tensor("wt", [_L - 1, 128, 32 * 128], f16,
                        kind="ExternalInput")
    ident = nc.dram_tensor("ident", [128, 128], f16, kind="ExternalInput")
    x0 = nc.dram_tensor("x0", [128, 16], f16, kind="ExternalInput")
    bias_mm = nc.dram_tensor("bias_mm", [1, _NBIAS * 128 + 1], f16,
                             kind="ExternalInput")
    out = nc.dram_tensor("out", [128, 2], f32, kind="ExternalOutput")

    with tile.TileContext(nc) as tc:
        with (
            tc.tile_pool(name="w0", bufs=1) as w0pool,
            tc.tile_pool(name="w", bufs=_L - 1) as wpool,
            tc.tile_pool(name="g", bufs=_L) as gpool,
            tc.tile_pool(name="src", bufs=4) as srcpool,
            tc.tile_pool(name="sg", bufs=4) as sgpool,
            tc.tile_pool(name="c", bufs=1) as cpool,
            tc.tile_pool(name="ps", bufs=3, space="PSUM") as pspool,
            tc.tile_pool(name="psf", bufs=1, space="PSUM") as psfpool,
            tc.tile_pool(name="pst", bufs=2, space="PSUM") as pstpool,
            tc.tile_pool(name="pgt", bufs=2, space="PSUM") as pgtpool,
            tc.tile_pool(name="dram", bufs=2 * _L, space="DRAM") as dpool,
        ):
            bias_t = cpool.tile([1, _NBIAS * 128 + 1], f16)
            nc.sync.dma_start(bias_t[:], bias_mm.ap())
            ident_t = cpool.tile([128, 128], f16)
            nc.sync.dma_start(ident_t[:], ident.ap())

            gath = [gpool.tile([128, 16], f16, tag="g", name=f"gath{i}")
                    for i in range(_L)]
            nc.sync.dma_start(gath[0][:], x0.ap())

            w0sb = w0pool.tile([128, 256 * 128], f16)
            nc.sync.dma_start(w0sb[:], wt0.ap())
            wsb = [None]
            for i in range(1, _L):
                w = wpool.tile([128, 32 * 128], f16, tag="w")
                nc.sync.dma_start(w[:], wt.ap()[i - 1])
                wsb.append(w)

            ones = bias_t[:, _ONES_COL:_ONES_COL + 1]

            # ---- layer 0: full 2048x2048 matvec on every core ----
            # One start/stop pair per 2KB PSUM zero region: start=True zeroes
            # the WHOLE region, so only the very first matmul starts and only
            # the very last stops; per-address first-touch semantics handle
            # the other columns.
            psf = psfpool.tile([128, 16], f32)
            for kt in range(16):
                for mo in range(16):
                    c = kt * 16 + mo
                    nc.tensor.matmul(
                        psf[:, mo:mo + 1],
                        lhsT=w0sb[:, c * 128:(c + 1) * 128],
                        rhs=gath[0][:, kt:kt + 1],
                        start=(kt == 0 and mo == 0), stop=False,
                    )
            for mo in range(16):
                c0 = (28 + mo) * 128
                nc.tensor.matmul(
                    psf[:, mo:mo + 1],
                    lhsT=bias_t[:, c0:c0 + 128],
                    rhs=ones,
                    start=False, stop=(mo == 15),
                )
            sgf = sgpool.tile([128, 16], f32, tag="sgf")
            nc.scalar.activation(sgf[:], psf[:], AF.Sigmoid,
                                 scale=float(_S[1]))
            nc.vector.tensor_mul(gath[1][:], psf[:], sgf[:])

            # ---- layers 1..14: row-sharded with AllGather between ----
            for i in range(1, _L):
                ps = pspool.tile([128, 2], f32, tag="ps", name=f"ps{i}")
                for mt in range(2):
                    for j in range(_M):
                        for mo in range(2):
                            c = (mt * 8 + j) * 2 + mo
                            nc.tensor.matmul(
                                ps[:, mo:mo + 1],
                                lhsT=wsb[i][:, c * 128:(c + 1) * 128],
                                rhs=gath[i][:, 2 * j + mt:2 * j + mt + 1],
                                start=(mt == 0 and j == 0 and mo == 0),
                                stop=False,
                            )
                for mo in range(2):
                    c0 = ((i - 1) * 2 + mo) * 128
                    nc.tensor.matmul(
                        ps[:, mo:mo + 1],
                        lhsT=bias_t[:, c0:c0 + 128],
                        rhs=ones,
                        start=False, stop=(mo == 1),
                    )
                if i < _L - 1:
                    src = srcpool.tile([128, 2], f16, tag="src")
                    sg = sgpool.tile([128, 2], f32, tag="sg")
                    # psum = y_i/S[i+1] (bias folded); sg = sigmoid(y_i)
                    nc.scalar.activation(sg[:], ps[:], AF.Sigmoid,
                                         scale=float(_S[i + 1]))
                    nc.vector.tensor_mul(src[:], ps[:], sg[:])
                    # PE-transpose [128,2] -> [2,128] so comm DMAs stay
                    # per-partition contiguous.
                    srcT = pstpool.tile([2, 128], f16, tag="pst",
                                        name=f"srcT{i}")
                    nc.tensor.transpose(srcT[:], src[:], ident_t[:])
                    srcTs = srcpool.tile([2, 128], f16, tag="srcTs")
                    nc.vector.tensor_copy(srcTs[:], srcT[:])
                    cc_in = dpool.tile([2, 128], f16, tag="ccin")
                    nc.scalar.dma_start(cc_in[:], srcTs[:])
                    cc_out = dpool.tile([16, 128], f16, tag="ccout")
                    nc.gpsimd.collective_compute(
                        "AllGather",
                        mybir.AluOpType.bypass,
                        replica_groups=[list(range(_M))],
                        ins=[cc_in.opt()],
                        outs=[cc_out.opt()],
                    )
                    rdraw = srcpool.tile([16, 128], f16, tag="rdraw")
                    nc.scalar.dma_start(rdraw[:], cc_out[:])
                    gT = pgtpool.tile([128, 16], f16, tag="gT",
                                      name=f"gT{i}")
                    nc.tensor.transpose(gT[:], rdraw[:], ident_t[:16, :16])
                    nc.vector.tensor_copy(gath[i + 1][:], gT[:])
                else:
                    o = srcpool.tile([128, 2], f32, tag="o")
                    # out = S[14] * psum  (bias already folded into psum)
                    nc.scalar.activation(o[:], ps[:], AF.Identity,
                                         scale=float(_S[14]))
                    nc.sync.dma_start(out.ap(), o[:])

    nc.compile()
    return nc


def _prep_inputs(x, W, b):
    """Host-side scaling, transposition and per-core slicing."""
    x = np.asarray(x, np.float32)
    W = np.asarray(W, np.float32)
    b = np.asarray(b, np.float32)
    S = _S

    Wf = np.empty_like(W)
    for i in range(_L - 1):
        Wf[i] = W[i] * (S[i] / S[i + 1])
    Wf[_L - 1] = W[_L - 1]  # folded with S15 = S14

    # layer-0 full: [pk, kt, mo, pm] -> col (kt*16+mo)*128+pm
    W0 = Wf[0].reshape(16, 128, 16, 128)        # [mo, pm, kt, pk]
    wt0 = np.ascontiguousarray(
        W0.transpose(3, 2, 0, 1).reshape(128, 256 * 128)).astype(np.float16)

    # layers 1..14 sharded: Wv[i, rm, mo, pm, ks, mt, pk]
    Wv = Wf[1:].reshape(_L - 1, _M, 2, 128, _M, 2, 128)
    xv = x.reshape(_M, 2, 128)
    # x0[pk, 2j+mt] = x[j*256 + mt*128 + pk]
    x0 = np.ascontiguousarray(
        xv.transpose(2, 0, 1).reshape(128, 16)).astype(np.float16)
    identity = np.eye(128, dtype=np.float16)

    in_maps = []
    for r in range(_M):
        Wc = Wv[:, r]                           # [i, mo, pm, j, mt, pk]
        Wc = Wc.transpose(0, 5, 4, 3, 1, 2)     # [i, pk, mt, j, mo, pm]
        wt_r = np.ascontiguousarray(
            Wc.reshape(_L - 1, 128, 32 * 128)).astype(np.float16)
        # bias_mm: layers 1..14 pairs, then layer-0 full, then 1.0
        bias = np.zeros(_NBIAS * 128 + 1, np.float32)
        for i in range(1, _L):
            s = S[i + 1] if i < _L - 1 else S[_L - 1]
            for mo in range(2):
                c0 = ((i - 1) * 2 + mo) * 128
                bias[c0:c0 + 128] = b[i, r * 256 + mo * 128:
                                      r * 256 + (mo + 1) * 128] / s
        for mo in range(16):
            c0 = (28 + mo) * 128
            bias[c0:c0 + 128] = b[0, mo * 128:(mo + 1) * 128] / S[1]
        bias[_ONES_COL] = 1.0
        in_maps.append({"wt0": wt0, "wt": wt_r, "x0": x0, "ident": identity,
                        "bias_mm": bias.reshape(1, -1).astype(np.float16)})
    return in_maps


def kernel(x, W, b, _trace=False):
    from concourse.bass_utils import run_bass_kernel_spmd

    key = "nc"
    if key not in _CACHE:
        _CACHE[key] = _build()
    nc = _CACHE[key]

    in_maps = _prep_inputs(x, W, b)
    res = run_bass_kernel_spmd(
        nc, in_maps, core_ids=list(range(_M)), trace=_trace)
    _CACHE["last_results"] = res
    return np.concatenate(
        [res.results[r]["out"].T.reshape(_SH) for r in range(_M)])



# revision 3
# speedup vs baseline: 1.1735x; 1.1735x over previous
"""Trainium2 Bass kernel for a 16-layer fully-connected chain (matvec per layer).

Computation (reference):
    v = x                       # [2048]
    for i in 0..13:  v = silu(W[i] @ v + b[i])
    out = W[14] @ v + b[14]

Design (8 NeuronCores, one trn2 chip) -- row/col alternation:
  - EVEN layers (0,2,...,14) are ROW-sharded: core r computes output
    neurons [r*256, (r+1)*256) from the FULL activation vector
    (weights-stationary matmuls, outputs across partitions).  The local
    256-block output feeds the next layer directly -- no communication.
  - ODD layers (1,3,...,13) are COL-sharded: core r computes a PARTIAL
    full 2048-vector using only ITS 256 activations (which the previous
    row layer just produced locally).  Partials are summed with ONE
    fp16 AllReduce per odd layer -- 7 collectives total instead of the
    13 AllGathers a pure row-sharded scheme needs.
  - A dummy 64B AllReduce is issued at t=0 so the ~12-17us ncfw
    collective-init barrier overlaps the initial weight DMA stream
    instead of stalling the first real collective.
  - Weight DMAs are issued in execution order (1 MB per layer per core,
    15 MB total) so layer 0 can start after ~3us instead of waiting for
    a replicated 8 MB layer-0 block.
  - fp16 weights+activations with per-layer power-of-2 activation
    scaling folded into the weights host-side (activations grow
    ~3.2x/layer to ~3e7, far beyond fp16 range): psum = y_i/S[i+1],
    sg = sigmoid(S[i+1]*psum), v' = psum*sg (silu is not
    scale-equivariant).  Final layer: out = S[14]*psum in fp32.
  - Biases ride into PSUM via rank-1 matmuls issued BEFORE the weight
    matmuls (they execute during the preceding epilogue's engine hops,
    off the critical path).  Col-layer biases are pre-divided by 8 so
    the AllReduce sums them back to 1x.
"""

import numpy as np

_L = 15        # number of weight matrices
_N = 2048      # neurons per layer
_M = 8         # cores
_SH = _N // _M  # 256 outputs per core

# S[i] = scale of the stored activation entering layer i (v_i = S[i]*v'_i).
_S = [1.0, 0.5, 1.0, 4.0, 8.0, 32.0, 128.0, 256.0, 1024.0, 2048.0,
      8192.0, 16384.0, 65536.0, 262144.0, 524288.0]

_NR = 8   # row layers: 0,2,...,14
_NC = 7   # col layers: 1,3,...,13

_ONES_COL = _NR * 2 * 128  # trailing 1.0 in br

_CACHE = {}


def _build():
    import concourse.bacc as bacc
    import concourse.mybir as mybir
    import concourse.tile as tile

    f32 = mybir.dt.float32
    f16 = mybir.dt.float16
    AF = mybir.ActivationFunctionType

    nc = bacc.Bacc("TRN2", target_bir_lowering=False, debug=False,
                   num_devices=_M)

    # row layers (l=2s): wr[s][pk, (kt*2+mo)*128+pm] =
    #   W'[2s][r*256+mo*128+pm, kt*128+pk]
    wr = nc.dram_tensor("wr", [_NR, 128, 32 * 128], f16, kind="ExternalInput")
    # col layers (l=2s+1): wc[s][pk, (kt*16+mt)*128+pm] =
    #   W'[2s+1][mt*128+pm, r*256+kt*128+pk]
    wc = nc.dram_tensor("wc", [_NC, 128, 32 * 128], f16, kind="ExternalInput")
    ident = nc.dram_tensor("ident", [128, 128], f16, kind="ExternalInput")
    x0 = nc.dram_tensor("x0", [128, 16], f16, kind="ExternalInput")
    # br: row biases (8 layers x 2 x 128) + trailing 1.0
    br = nc.dram_tensor("br", [1, _ONES_COL + 1], f16, kind="ExternalInput")
    # bc: col biases / 8 (7 layers x 16 x 128)
    bc = nc.dram_tensor("bc", [1, _NC * 16 * 128], f16, kind="ExternalInput")
    out = nc.dram_tensor("out", [128, 2], f32, kind="ExternalOutput")

    with tile.TileContext(nc) as tc:
        with (
            tc.tile_pool(name="wrp", bufs=_NR) as wrpool,
            tc.tile_pool(name="wcp", bufs=_NC) as wcpool,
            tc.tile_pool(name="c", bufs=1) as cpool,
            tc.tile_pool(name="act", bufs=2) as apool,
            tc.tile_pool(name="ps", bufs=2, space="PSUM") as pspool,
            tc.tile_pool(name="pc", bufs=2, space="PSUM") as pcpool,
            tc.tile_pool(name="pt", bufs=2, space="PSUM") as ptpool,
            tc.tile_pool(name="pg", bufs=2, space="PSUM") as pgpool,
            tc.tile_pool(name="dram", bufs=_NC, space="DRAM") as dpool,
            tc.tile_pool(name="dumd", bufs=1, space="DRAM") as dumpool,
        ):
            # ---- dummy prewarm collective: absorb ncfw init at t=0 ----
            dum_in = dumpool.tile([1, 32], f16, tag="dumi")
            nc.scalar.dma_start(dum_in[:], ident.ap()[0:1, 0:32])
            dum_out = dumpool.tile([1, 32], f16, tag="dum")
            nc.gpsimd.collective_compute(
                "AllReduce",
                mybir.AluOpType.add,
                replica_groups=[list(range(_M))],
                ins=[dum_in.opt()],
                outs=[dum_out.opt()],
            )

            br_t = cpool.tile([1, _ONES_COL + 1], f16)
            nc.sync.dma_start(br_t[:], br.ap())
            bc_t = cpool.tile([1, _NC * 16 * 128], f16)
            nc.sync.dma_start(bc_t[:], bc.ap())
            ident_t = cpool.tile([128, 128], f16)
            nc.sync.dma_start(ident_t[:], ident.ap())
            x0_t = apool.tile([128, 16], f16, tag="v", name="x0_t")
            nc.sync.dma_start(x0_t[:], x0.ap())

            # weight DMAs in execution order
            wr_sb, wc_sb = [], []
            for s in range(_NR):
                w = wrpool.tile([128, 32 * 128], f16, tag="wr",
                                name=f"wr{s}")
                nc.sync.dma_start(w[:], wr.ap()[s])
                wr_sb.append(w)
                if s < _NC:
                    w = wcpool.tile([128, 32 * 128], f16, tag="wc",
                                    name=f"wc{s}")
                    nc.sync.dma_start(w[:], wc.ap()[s])
                    wc_sb.append(w)

            ones = br_t[:, _ONES_COL:_ONES_COL + 1]

            v = x0_t
            for s in range(_NR):
                # ---- row layer l = 2s: 256 local outputs ----
                ps = pspool.tile([128, 2], f32, tag="ps", name=f"ps{s}")
                for mo in range(2):
                    c0 = (s * 2 + mo) * 128
                    nc.tensor.matmul(
                        ps[:, mo:mo + 1],
                        lhsT=br_t[:, c0:c0 + 128],
                        rhs=ones,
                        start=(mo == 0), stop=False,
                    )
                for kt in range(16):
                    for mo in range(2):
                        c = kt * 2 + mo
                        nc.tensor.matmul(
                            ps[:, mo:mo + 1],
                            lhsT=wr_sb[s][:, c * 128:(c + 1) * 128],
                            rhs=v[:, kt:kt + 1],
                            start=False, stop=(kt == 15 and mo == 1),
                        )
                if s == _NR - 1:
                    o = apool.tile([128, 2], f32, tag="o")
                    nc.scalar.activation(o[:], ps[:], AF.Identity,
                                         scale=float(_S[14]))
                    nc.sync.dma_start(out.ap(), o[:])
                    break

                sg2 = apool.tile([128, 2], f32, tag="sg2")
                nc.scalar.activation(sg2[:], ps[:], AF.Sigmoid,
                                     scale=float(_S[2 * s + 1]))
                vloc = apool.tile([128, 2], f16, tag="vloc")
                nc.vector.tensor_mul(vloc[:], ps[:], sg2[:])

                # ---- col layer l = 2s+1: partial full 2048-vector ----
                pc = pcpool.tile([128, 16], f32, tag="pc", name=f"pc{s}")
                for mt in range(16):
                    c0 = (s * 16 + mt) * 128
                    nc.tensor.matmul(
                        pc[:, mt:mt + 1],
                        lhsT=bc_t[:, c0:c0 + 128],
                        rhs=ones,
                        start=(mt == 0), stop=False,
                    )
                for kt in range(2):
                    for mt in range(16):
                        c = kt * 16 + mt
                        nc.tensor.matmul(
                            pc[:, mt:mt + 1],
                            lhsT=wc_sb[s][:, c * 128:(c + 1) * 128],
                            rhs=vloc[:, kt:kt + 1],
                            start=False, stop=(kt == 1 and mt == 15),
                        )
                # psum [128,16] -> sbuf f16 -> PE transpose -> [16,128]
                sc = apool.tile([128, 16], f16, tag="sc")
                nc.vector.tensor_copy(sc[:], pc[:])
                pt = ptpool.tile([16, 128], f16, tag="pt", name=f"pt{s}")
                nc.tensor.transpose(pt[:], sc[:], ident_t[:])
                scs = apool.tile([16, 128], f16, tag="scs")
                nc.vector.tensor_copy(scs[:], pt[:])
                cin = dpool.tile([16, 128], f16, tag="cin")
                nc.scalar.dma_start(cin[:], scs[:])
                cout = dpool.tile([16, 128], f16, tag="cout",
                                  addr_space="Shared")
                nc.gpsimd.collective_compute(
                    "AllReduce",
                    mybir.AluOpType.add,
                    replica_groups=[list(range(_M))],
                    ins=[cin.opt()],
                    outs=[cout.opt()],
                )
                y16 = apool.tile([16, 128], f16, tag="y16")
                nc.scalar.dma_start(y16[:], cout[:])
                sg16 = apool.tile([16, 128], f32, tag="sg16")
                nc.scalar.activation(sg16[:], y16[:], AF.Sigmoid,
                                     scale=float(_S[2 * s + 2]))
                u16 = apool.tile([16, 128], f16, tag="u16")
                nc.vector.tensor_mul(u16[:], y16[:], sg16[:])
                pg = pgpool.tile([128, 16], f16, tag="pg", name=f"pg{s}")
                nc.tensor.transpose(pg[:], u16[:], ident_t[:16, :16])
                v = apool.tile([128, 16], f16, tag="v", name=f"v{s}")
                nc.vector.tensor_copy(v[:], pg[:])

    nc.compile()
    return nc


def _prep_inputs(x, W, b):
    """Host-side scaling, transposition and per-core slicing."""
    x = np.asarray(x, np.float32)
    W = np.asarray(W, np.float32)
    b = np.asarray(b, np.float32)
    S = _S

    Wf = np.empty_like(W)
    for i in range(_L - 1):
        Wf[i] = W[i] * (S[i] / S[i + 1])
    Wf[_L - 1] = W[_L - 1]  # folded with S15 = S14

    # x0[p, t] = x[t*128+p]
    x0 = np.ascontiguousarray(x.reshape(16, 128).T).astype(np.float16)
    identity = np.eye(128, dtype=np.float16)

    in_maps = []
    for r in range(_M):
        # row layers l=2s
        wr = np.empty((_NR, 128, 32 * 128), np.float16)
        for s in range(_NR):
            A = Wf[2 * s][r * 256:(r + 1) * 256, :]       # [256, 2048]
            A = A.reshape(2, 128, 16, 128)                # [mo, pm, kt, pk]
            wr[s] = A.transpose(3, 2, 0, 1).reshape(128, 32 * 128)
        # col layers l=2s+1
        wcv = np.empty((_NC, 128, 32 * 128), np.float16)
        for s in range(_NC):
            B = Wf[2 * s + 1][:, r * 256:(r + 1) * 256]   # [2048, 256]
            B = B.reshape(16, 128, 2, 128)                # [mt, pm, kt, pk]
            wcv[s] = B.transpose(3, 2, 0, 1).reshape(128, 32 * 128)
        # br: row biases + trailing 1.0
        brv = np.zeros(_ONES_COL + 1, np.float32)
        for s in range(_NR):
            l = 2 * s
            sc = S[l + 1] if l < _L - 1 else S[_L - 1]
            blk = b[l, r * 256:(r + 1) * 256] / sc
            brv[s * 256:(s + 1) * 256] = blk
        brv[_ONES_COL] = 1.0
        # bc: col biases / 8
        bcv = np.zeros(_NC * 16 * 128, np.float32)
        for s in range(_NC):
            l = 2 * s + 1
            bcv[s * 2048:(s + 1) * 2048] = b[l] / (8.0 * S[l + 1])
        in_maps.append({
            "wr": wr, "wc": wcv, "x0": x0, "ident": identity,
            "br": brv.reshape(1, -1).astype(np.float16),
            "bc": bcv.reshape(1, -1).astype(np.float16),
        })
    return in_maps


def kernel(x, W, b, _trace=False):
    from concourse.bass_utils import run_bass_kernel_spmd

    key = "nc"
    if key not in _CACHE:
        _CACHE[key] = _build()
    nc = _CACHE[key]

    in_maps = _prep_inputs(x, W, b)
    res = run_bass_kernel_spmd(
        nc, in_maps, core_ids=list(range(_M)), trace=_trace)
    _CACHE["last_results"] = res
    return np.concatenate(
        [res.results[r]["out"].T.reshape(_SH) for r in range(_M)])


# revision 4
# speedup vs baseline: 1.2577x; 1.0718x over previous
"""Trainium2 Bass kernel for a 16-layer fully-connected chain (matvec per layer).

Computation (reference):
    v = x                       # [2048]
    for i in 0..13:  v = silu(W[i] @ v + b[i])
    out = W[14] @ v + b[14]

Design (8 NeuronCores, one trn2 chip) -- v3:
  - A fixed ~79us ncfw collective-init window gates the FIRST collective
    regardless of dependencies (measured), and cores launch with up to
    ~27us skew.  Layers 0-1 are therefore computed REPLICATED (full
    2048x2048 matvec on every core, 8 MB fp16 weights each, streamed as
    2x4MB double-buffered chunks) -- ~50us of comm-free local work that
    hides inside the init window on every core.
  - Layers 2..14 alternate ROW-sharded (even: core r computes outputs
    [r*256,(r+1)*256) from the full vector; no comm) and COL-sharded
    (odd: core r computes a PARTIAL full 2048-vector from its local 256
    activations).  Partials are combined with an AllGather (4.6us floor
    vs AllReduce's 9.7us) + a one-instruction PE selector-matmul that
    sums the 8 partials on-core.  6 collectives total (vs 13 for pure
    row sharding).
  - fp16 weights+activations with per-layer power-of-2 activation
    scaling folded into the weights host-side: psum = y_i/S[i+1],
    sg = sigmoid(S[i+1]*psum), v' = psum*sg.  Final layer (14, row):
    out = S[14]*psum in fp32.
  - Biases enter PSUM via rank-1 matmuls issued BEFORE the weight
    matmuls (they run during the preceding epilogue, off the critical
    path).  Col-layer biases are pre-divided by 8 so the partial-sum
    reduce restores them to 1x.
"""

import numpy as np

_L = 15        # number of weight matrices
_N = 2048      # neurons per layer
_M = 8         # cores
_SH = _N // _M  # 256 outputs per core

# S[i] = scale of the stored activation entering layer i (v_i = S[i]*v'_i).
_S = [1.0, 0.5, 1.0, 4.0, 8.0, 32.0, 128.0, 256.0, 1024.0, 2048.0,
      8192.0, 16384.0, 65536.0, 262144.0, 524288.0]

_NF = 2   # replicated (full) layers: 0,1
_NR = 7   # row layers: 2,4,...,14
_NC = 6   # col layers: 3,5,...,13

# bias tensor segment offsets (in columns of 1 partition)
_OFF_REPL = 0                      # 2 x 16 x 128
_OFF_ROW = _NF * 2048              # 7 x 2 x 128
_OFF_COL = _OFF_ROW + _NR * 256    # 6 x 16 x 128
_OFF_ONES = _OFF_COL + _NC * 2048  # single 1.0
_NB = _OFF_ONES + 1

_CACHE = {}


def _build():
    import concourse.bacc as bacc
    import concourse.mybir as mybir
    import concourse.tile as tile

    f32 = mybir.dt.float32
    f16 = mybir.dt.float16
    AF = mybir.ActivationFunctionType

    nc = bacc.Bacc("TRN2", target_bir_lowering=False, debug=False,
                   num_devices=_M)

    # replicated layers 0,1 (full): col (kt*16 + mo)*128 + pm holds
    # W'[k][mo*128+pm, kt*128+pk] at partition pk; split in 2 chunks of
    # 128 col-blocks (kt 0..7 | 8..15).
    wf = nc.dram_tensor("wf", [_NF * 2, 128, 128 * 128], f16,
                        kind="ExternalInput")
    # row layers l=2+2s: wr[s][pk, (kt*2+mo)*128+pm] =
    #   W'[l][r*256+mo*128+pm, kt*128+pk]
    wr = nc.dram_tensor("wr", [_NR, 128, 32 * 128], f16,
                        kind="ExternalInput")
    # col layers l=3+2s: wc[s][pk, (kt*16+mt)*128+pm] =
    #   W'[l][mt*128+pm, r*256+kt*128+pk]
    wc = nc.dram_tensor("wc", [_NC, 128, 32 * 128], f16,
                        kind="ExternalInput")
    ident = nc.dram_tensor("ident", [128, 128], f16, kind="ExternalInput")
    # selm[q, t] = 1 if q % 16 == t  (sums 8 gathered partials on PE)
    selm = nc.dram_tensor("selm", [128, 16], f16, kind="ExternalInput")
    x0 = nc.dram_tensor("x0", [128, 16], f16, kind="ExternalInput")
    bb = nc.dram_tensor("bb", [1, _NB], f16, kind="ExternalInput")
    out = nc.dram_tensor("out", [128, 2], f32, kind="ExternalOutput")

    with tile.TileContext(nc) as tc:
        with (
            tc.tile_pool(name="wfp", bufs=2) as wfpool,
            tc.tile_pool(name="wrp", bufs=_NR) as wrpool,
            tc.tile_pool(name="wcp", bufs=_NC) as wcpool,
            tc.tile_pool(name="c", bufs=1) as cpool,
            tc.tile_pool(name="act", bufs=2) as apool,
            tc.tile_pool(name="ps", bufs=1, space="PSUM") as pspool,
            tc.tile_pool(name="pc", bufs=2, space="PSUM") as pcpool,
            tc.tile_pool(name="pt", bufs=1, space="PSUM") as ptpool,
            tc.tile_pool(name="pr", bufs=1, space="PSUM") as prpool,
            tc.tile_pool(name="pg", bufs=1, space="PSUM") as pgpool,
            tc.tile_pool(name="dram", bufs=_NC, space="DRAM") as dpool,
        ):
            bb_t = cpool.tile([1, _NB], f16)
            nc.sync.dma_start(bb_t[:], bb.ap())
            ident_t = cpool.tile([128, 128], f16)
            nc.sync.dma_start(ident_t[:], ident.ap())
            selm_t = cpool.tile([128, 16], f16)
            nc.sync.dma_start(selm_t[:], selm.ap())
            x0_t = apool.tile([128, 16], f16, tag="v", name="x0_t")
            nc.sync.dma_start(x0_t[:], x0.ap())

            # replicated-layer weight chunks (double-buffered 4MB slots)
            wf_sb = []
            for c in range(_NF * 2):
                w = wfpool.tile([128, 128 * 128], f16, tag="wf",
                                name=f"wf{c}")
                nc.sync.dma_start(w[:], wf.ap()[c])
                wf_sb.append(w)
            # sharded weights, interleaved in execution order
            wr_sb, wc_sb = [], []
            for s in range(_NR):
                w = wrpool.tile([128, 32 * 128], f16, tag="wr",
                                name=f"wr{s}")
                nc.sync.dma_start(w[:], wr.ap()[s])
                wr_sb.append(w)
                if s < _NC:
                    w = wcpool.tile([128, 32 * 128], f16, tag="wc",
                                    name=f"wc{s}")
                    nc.sync.dma_start(w[:], wc.ap()[s])
                    wc_sb.append(w)

            ones = bb_t[:, _OFF_ONES:_OFF_ONES + 1]

            # ---- replicated layers 0,1: full matvec on every core ----
            v = x0_t
            for k in range(_NF):
                pf = pcpool.tile([128, 16], f32, tag="pc", name=f"pf{k}")
                for mo in range(16):
                    c0 = (_OFF_REPL + k * 2048) + mo * 128
                    nc.tensor.matmul(
                        pf[:, mo:mo + 1],
                        lhsT=bb_t[:, c0:c0 + 128],
                        rhs=ones,
                        start=(mo == 0), stop=False,
                    )
                for h in range(2):
                    wch = wf_sb[k * 2 + h]
                    for ktl in range(8):
                        for mo in range(16):
                            c = ktl * 16 + mo
                            nc.tensor.matmul(
                                pf[:, mo:mo + 1],
                                lhsT=wch[:, c * 128:(c + 1) * 128],
                                rhs=v[:, h * 8 + ktl:h * 8 + ktl + 1],
                                start=False,
                                stop=(h == 1 and ktl == 7 and mo == 15),
                            )
                sgf = apool.tile([128, 16], f32, tag="sgf", name=f"sgf{k}")
                nc.scalar.activation(sgf[:], pf[:], AF.Sigmoid,
                                     scale=float(_S[k + 1]))
                vn = apool.tile([128, 16], f16, tag="v", name=f"vf{k}")
                nc.vector.tensor_mul(vn[:], pf[:], sgf[:])
                v = vn

            # ---- layers 2..14: row/col alternation, AG after cols ----
            for s in range(_NR):
                l = 2 + 2 * s
                ps = pspool.tile([128, 2], f32, tag="ps", name=f"ps{s}")
                for mo in range(2):
                    c0 = _OFF_ROW + (s * 2 + mo) * 128
                    nc.tensor.matmul(
                        ps[:, mo:mo + 1],
                        lhsT=bb_t[:, c0:c0 + 128],
                        rhs=ones,
                        start=(mo == 0), stop=False,
                    )
                for kt in range(16):
                    for mo in range(2):
                        c = kt * 2 + mo
                        nc.tensor.matmul(
                            ps[:, mo:mo + 1],
                            lhsT=wr_sb[s][:, c * 128:(c + 1) * 128],
                            rhs=v[:, kt:kt + 1],
                            start=False, stop=(kt == 15 and mo == 1),
                        )
                if s == _NR - 1:
                    o = apool.tile([128, 2], f32, tag="o")
                    nc.scalar.activation(o[:], ps[:], AF.Identity,
                                         scale=float(_S[14]))
                    nc.sync.dma_start(out.ap(), o[:])
                    break

                sg2 = apool.tile([128, 2], f32, tag="sg2")
                nc.scalar.activation(sg2[:], ps[:], AF.Sigmoid,
                                     scale=float(_S[l + 1]))
                vloc = apool.tile([128, 2], f16, tag="vloc")
                nc.vector.tensor_mul(vloc[:], ps[:], sg2[:])

                # col layer l+1 = 3+2s: partial full 2048-vector
                pc = pcpool.tile([128, 16], f32, tag="pc", name=f"pc{s}")
                for mt in range(16):
                    c0 = _OFF_COL + (s * 16 + mt) * 128
                    nc.tensor.matmul(
                        pc[:, mt:mt + 1],
                        lhsT=bb_t[:, c0:c0 + 128],
                        rhs=ones,
                        start=(mt == 0), stop=False,
                    )
                for kt in range(2):
                    for mt in range(16):
                        c = kt * 16 + mt
                        nc.tensor.matmul(
                            pc[:, mt:mt + 1],
                            lhsT=wc_sb[s][:, c * 128:(c + 1) * 128],
                            rhs=vloc[:, kt:kt + 1],
                            start=False, stop=(kt == 1 and mt == 15),
                        )
                # psum [128,16] -> sbuf f16 -> PE transpose -> [16,128]
                sc = apool.tile([128, 16], f16, tag="sc")
                nc.vector.tensor_copy(sc[:], pc[:])
                pt = ptpool.tile([16, 128], f16, tag="pt", name=f"pt{s}")
                nc.tensor.transpose(pt[:], sc[:], ident_t[:])
                scs = apool.tile([16, 128], f16, tag="scs")
                nc.vector.tensor_copy(scs[:], pt[:])
                cin = dpool.tile([16, 128], f16, tag="cin")
                nc.scalar.dma_start(cin[:], scs[:])
                cout = dpool.tile([128, 128], f16, tag="cout",
                                  addr_space="Shared")
                nc.gpsimd.collective_compute(
                    "AllGather",
                    mybir.AluOpType.bypass,
                    replica_groups=[list(range(_M))],
                    ins=[cin.opt()],
                    outs=[cout.opt()],
                )
                ccsb = apool.tile([128, 128], f16, tag="ccsb")
                nc.scalar.dma_start(ccsb[:], cout[:])
                # sum the 8 partials: y16[t,p] = sum_r ccsb[16r+t, p]
                pr = prpool.tile([16, 128], f32, tag="pr", name=f"pr{s}")
                nc.tensor.matmul(pr[:], lhsT=selm_t[:], rhs=ccsb[:],
                                 start=True, stop=True)
                sg16 = apool.tile([16, 128], f32, tag="sg16")
                nc.scalar.activation(sg16[:], pr[:], AF.Sigmoid,
                                     scale=float(_S[l + 2]))
                u16 = apool.tile([16, 128], f16, tag="u16")
                nc.vector.tensor_mul(u16[:], pr[:], sg16[:])
                pg = pgpool.tile([128, 16], f16, tag="pg", name=f"pg{s}")
                nc.tensor.transpose(pg[:], u16[:], ident_t[:16, :16])
                v = apool.tile([128, 16], f16, tag="v", name=f"v{s}")
                nc.vector.tensor_copy(v[:], pg[:])

    nc.compile()
    return nc


def _prep_inputs(x, W, b):
    """Host-side scaling, transposition and per-core slicing."""
    x = np.asarray(x, np.float32)
    W = np.asarray(W, np.float32)
    b = np.asarray(b, np.float32)
    S = _S

    Wf = np.empty_like(W)
    for i in range(_L - 1):
        Wf[i] = W[i] * (S[i] / S[i + 1])
    Wf[_L - 1] = W[_L - 1]  # folded with S15 = S14

    # replicated layers 0,1 (shared by all cores)
    wfv = np.empty((_NF * 2, 128, 128 * 128), np.float16)
    for k in range(_NF):
        A = Wf[k].reshape(16, 128, 16, 128)        # [mo, pm, kt, pk]
        A = A.transpose(3, 2, 0, 1)                # [pk, kt, mo, pm]
        wfv[2 * k] = A[:, :8].reshape(128, 128 * 128)
        wfv[2 * k + 1] = A[:, 8:].reshape(128, 128 * 128)

    # x0[p, t] = x[t*128+p]
    x0 = np.ascontiguousarray(x.reshape(16, 128).T).astype(np.float16)
    identity = np.eye(128, dtype=np.float16)
    selm = np.tile(np.eye(16, dtype=np.float16), (8, 1))

    in_maps = []
    for r in range(_M):
        wrv = np.empty((_NR, 128, 32 * 128), np.float16)
        for s in range(_NR):
            l = 2 + 2 * s
            A = Wf[l][r * 256:(r + 1) * 256, :]    # [256, 2048]
            A = A.reshape(2, 128, 16, 128)         # [mo, pm, kt, pk]
            wrv[s] = A.transpose(3, 2, 0, 1).reshape(128, 32 * 128)
        wcv = np.empty((_NC, 128, 32 * 128), np.float16)
        for s in range(_NC):
            l = 3 + 2 * s
            B = Wf[l][:, r * 256:(r + 1) * 256]    # [2048, 256]
            B = B.reshape(16, 128, 2, 128)         # [mt, pm, kt, pk]
            wcv[s] = B.transpose(3, 2, 0, 1).reshape(128, 32 * 128)
        bbv = np.zeros(_NB, np.float32)
        for k in range(_NF):
            bbv[_OFF_REPL + k * 2048:_OFF_REPL + (k + 1) * 2048] = \
                b[k] / S[k + 1]
        for s in range(_NR):
            l = 2 + 2 * s
            sc = S[l + 1] if l < _L - 1 else S[_L - 1]
            bbv[_OFF_ROW + s * 256:_OFF_ROW + (s + 1) * 256] = \
                b[l, r * 256:(r + 1) * 256] / sc
        for s in range(_NC):
            l = 3 + 2 * s
            bbv[_OFF_COL + s * 2048:_OFF_COL + (s + 1) * 2048] = \
                b[l] / (8.0 * S[l + 1])
        bbv[_OFF_ONES] = 1.0
        in_maps.append({
            "wf": wfv, "wr": wrv, "wc": wcv, "x0": x0, "ident": identity,
            "selm": selm,
            "bb": bbv.reshape(1, -1).astype(np.float16),
        })
    return in_maps


def kernel(x, W, b, _trace=False):
    from concourse.bass_utils import run_bass_kernel_spmd

    key = "nc"
    if key not in _CACHE:
        _CACHE[key] = _build()
    nc = _CACHE[key]

    in_maps = _prep_inputs(x, W, b)
    res = run_bass_kernel_spmd(
        nc, in_maps, core_ids=list(range(_M)), trace=_trace)
    _CACHE["last_results"] = res
    return np.concatenate(
        [res.results[r]["out"].T.reshape(_SH) for r in range(_M)])
